# revision 5
# baseline (speedup 1.0000x reference)
"""Causal GQA attention (B=4, S=1024, H=16 q-heads, 4 kv-heads, D=128) on 8 trn2 cores.

Sharding: 16 (batch, kv-group) pairs -> 2 pairs/core; each pair carries 4 query
heads, so each core runs 8 independent causal-attention head-units.

Per head-unit (transposed-scores formulation, S^T[sk, sq]):
  QK^T on PE in fp8(e4m3) DoubleRow perf mode (0.5 cyc/col): q is split
    hi+lo into the two DoubleRow k-tiles (near-exact q), k single fp8 via a
    zero-stride broadcast lhsT.  Diagonal blocks accumulate a second
    DoubleRow matmul with k_lo so the high-weight near-diagonal scores are
    computed with near-fp16 accuracy.
  The matmul inputs are pre-scaled so PSUM holds 1024*log2(e)*score:
    - ACT computes exact exp via activation(Exp, scale=ln2/1024)
    - DVE computes a one-op Schraudolph exp2: int16(psum + B16) bit-cast
      to fp16 (exponent-field arithmetic), ~+-3% spread which largely
      cancels in the softmax normalization.
  Diagonal blocks: one batched DVE scalar_tensor_tensor per head fusing
    (psum + B16) * causal_mask -> int16, writing all 8 diagonal 128x128
    P-blocks through a custom strided AP.
  PV on PE in fp16; parallel 1-col ones-matmuls accumulate the softmax
    denominator into spare PSUM columns.  PSUM holds unnormalized O and den;
    ACT/DVE evacuate both to SBUF as fp16 and the o/den division runs on
    host after the gather.

PSUM budget (8 banks x 2KB): scores 2x[128,896] (4 banks), dps [128,1024]
(2 banks, diagonal blocks then reused as scratch for blocks j=4..6 + den),
po [128,8,128] (2 banks).
"""

import os
import sys

for _p in ("/opt/trn_rl_repo", "/root/.axon_site/_ro/trn_rl_repo"):
    if os.path.isdir(_p) and _p not in sys.path:
        sys.path.insert(0, _p)

from contextlib import ExitStack

import numpy as np
import ml_dtypes

import concourse.bass as bass
import concourse.tile as tile
from concourse import bacc, mybir
from concourse.ap import AP
from concourse.bass_utils import run_bass_kernel_spmd

B = 4
S = 1024
H = 16
HKV = 4
G = H // HKV
D = 128
SCALE = 0.08838834764831845
LOG2E = 1.4426950408889634
NCORES = 8
PAIRS = (B * HKV) // NCORES  # 2
NU = PAIRS * G  # 8 head-units per core
NT = S // 128  # 8

# PSUM holds 1024*log2e*score; folded into host-side q/k prescale.
CSUM = 1024.0 * LOG2E * SCALE  # ~130.577
CK = 8.0
CQ = CSUM / CK
ACT_SCALE = 1.0 / (1024.0 * LOG2E)  # recovers score from psum for exact exp
B16 = 15361.0  # schraudolph exp2 bias (int16/fp16 domain)

FP16 = mybir.dt.float16
FP32 = mybir.dt.float32
FP8 = mybir.dt.float8e4
I16 = mybir.dt.int16
DR = mybir.MatmulPerfMode.DoubleRow

# scratch layout inside dps after the diagonal STT consumed it (fp32 cols)
SCRATCH = {4: 0, 5: 512, 6: 768}  # block j -> col offset (no bank straddle)
DEN_COL = 904
# exp engine per off-diag block j (True -> ACT exact exp, False -> DVE exp2)
ACT_BLK = [True, True, True, True, False, False, False]

_cache = {}


def build_program():
    nc = bacc.Bacc("TRN2", target_bir_lowering=False, debug=False, num_devices=NCORES)

    q8_d = nc.dram_tensor("q8", [NU, 128, 2, S], FP8, kind="ExternalInput").ap()
    k8_d = nc.dram_tensor("k8", [128, PAIRS, 2, S], FP8, kind="ExternalInput").ap()
    v16_d = nc.dram_tensor("v16", [128, PAIRS, NT, D], FP16, kind="ExternalInput").ap()
    mask_d = nc.dram_tensor("mask", [128, 128], FP16, kind="ExternalInput").ap()
    o_d = nc.dram_tensor("o", [NU, 128, NT, D], FP16, kind="ExternalOutput").ap()
    od_d = nc.dram_tensor("oden", [NU, 128, NT], FP16, kind="ExternalOutput").ap()

    with tile.TileContext(nc) as tc, ExitStack() as ctx:
        const = ctx.enter_context(tc.tile_pool(name="const", bufs=1))
        pt_pool = ctx.enter_context(tc.tile_pool(name="pt_pool", bufs=2))
        outp = ctx.enter_context(tc.tile_pool(name="outp", bufs=2))
        ps_s = ctx.enter_context(tc.tile_pool(name="ps_s", bufs=2, space="PSUM"))
        ps_d = ctx.enter_context(tc.tile_pool(name="ps_d", bufs=1, space="PSUM"))
        ps_o = ctx.enter_context(tc.tile_pool(name="ps_o", bufs=1, space="PSUM"))

        q8s = const.tile([128, NU, 2, S], FP8)
        k8s = const.tile([128, PAIRS, 2, S], FP8)
        v16s = const.tile([128, PAIRS, NT, D], FP16)
        mask16 = const.tile([128, 128], FP16)
        ones16 = const.tile([128, 1], FP16)

        nc.sync.dma_start(out=mask16, in_=mask_d)
        nc.sync.dma_start(out=k8s, in_=k8_d)
        nc.vector.memset(ones16, 1.0)
        nc.sync.dma_start(out=v16s[:, 0], in_=v16_d[:, 0])
        nc.sync.dma_start(out=q8s[:, 0], in_=q8_d[0])
        nc.sync.dma_start(out=q8s[:, 1], in_=q8_d[1])
        nc.sync.dma_start(out=v16s[:, 1], in_=v16_d[:, 1])
        for u in range(2, NU):
            nc.sync.dma_start(out=q8s[:, u], in_=q8_d[u])

        def k_lhsT(pair, j, lo):
            # [128, 2, 128] DoubleRow stationary: k block j broadcast over the
            # two k-tiles (matching the hi/lo q streams); lo picks k_hi/k_lo.
            blk = k8s[:, pair, lo, 128 * j : 128 * j + 128]
            return blk.rearrange("p (o k) -> p o k", o=1).broadcast_to([128, 2, 128])

        def head(u):
            pair, h = divmod(u, G)
            pt = pt_pool.tile([128, NT, S], FP16, tag="pt", name=f"pt_{u}")
            ob = outp.tile([128, NT, D], FP16, tag="ob", name=f"ob_{u}")
            obden = outp.tile([128, NT], FP16, tag="obd", name=f"obd_{u}")
            dps = ps_d.tile([128, 1024], FP32, tag="dps", name=f"dps_{u}")
            po = ps_o.tile([128, NT, D], FP32, tag="po", name=f"po_{u}")
            dpsv = dps.rearrange("p (j c) -> p j c", j=NT)

            qrhs = q8s[:, u]  # [128, 2, S]

            # --- diagonal QK blocks -> dps, with k_lo correction ---
            for j in range(NT):
                sq = 128 * j
                for lo in (0, 1):
                    nc.tensor.matmul(
                        dpsv[:, j, :], lhsT=k_lhsT(pair, j, lo),
                        rhs=qrhs[:, :, sq : sq + 128],
                        start=(lo == 0), stop=(lo == 1), perf_mode=DR,
                    )
            # one fused DVE op: (dps + B16) * mask -> int16 -> all 8 diagonal
            # P blocks via strided AP pt[:, j, 128j:128j+128]
            pt_i16 = pt.bitcast(I16)
            base = pt_i16[:, 0, 0:128]
            diag_out = AP(base.tensor, base.offset,
                          [list(base.ap[0]), [S + 128, NT], [1, 128]])
            mask_b = mask16.rearrange("p (o c) -> p o c", o=1).broadcast_to(
                [128, NT, 128])
            nc.vector.scalar_tensor_tensor(
                out=diag_out, in0=dpsv, scalar=B16, in1=mask_b,
                op0=mybir.AluOpType.add, op1=mybir.AluOpType.mult,
            )

            def pv(i):
                for j in range(i + 1):
                    lhsT = pt[:, j, 128 * i : 128 * i + 128]
                    nc.tensor.matmul(
                        po[:, i, :], lhsT=lhsT, rhs=v16s[:, pair, j, :],
                        start=(j == 0), stop=(j == i),
                    )
                    nc.tensor.matmul(
                        dps[:, DEN_COL + i : DEN_COL + i + 1], lhsT=lhsT,
                        rhs=ones16, start=(j == 0), stop=(j == i),
                    )

            def exp_piece(j, psrc, q0, w):
                if ACT_BLK[j]:
                    nc.scalar.activation(
                        out=pt[:, j, q0 : q0 + w], in_=psrc,
                        func=mybir.ActivationFunctionType.Exp, scale=ACT_SCALE,
                    )
                else:
                    nc.vector.tensor_scalar(
                        out=pt_i16[:, j, q0 : q0 + w], in0=psrc,
                        scalar1=B16, scalar2=None, op0=mybir.AluOpType.add,
                    )

            pv(0)

            # --- off-diag blocks j=0..3 through ps_s; j=4..6 through dps ---
            for j in range(NT - 1):
                q0 = 128 * (j + 1)
                w = S - q0
                if j < 4:
                    sp = ps_s.tile([128, 896], FP32, tag="sp", name=f"sp_{u}_{j}")
                    dst = sp[:, 0:w]
                else:
                    c0 = SCRATCH[j]
                    dst = dps[:, c0 : c0 + w]
                for c in range(0, w, 256):
                    cw = min(256, w - c)
                    nc.tensor.matmul(
                        dst[:, c : c + cw], lhsT=k_lhsT(pair, j, 0),
                        rhs=qrhs[:, :, q0 + c : q0 + c + cw],
                        start=True, stop=True, perf_mode=DR,
                    )
                exp_piece(j, dst, q0, w)
                pv(j + 1)
                if j == 3:
                    # rowtiles 0-3 complete: ACT evacuates the first half of po
                    nc.scalar.copy(ob[:, 0:4, :], po[:, 0:4, :])

            # --- evacuate second half + den; host divides o by den ---
            nc.vector.tensor_copy(obden, dps[:, DEN_COL : DEN_COL + NT])
            nc.vector.tensor_copy(ob[:, 4:8, :], po[:, 4:8, :])
            nc.sync.dma_start(out=o_d[u], in_=ob)
            nc.sync.dma_start(out=od_d[u], in_=obden)

        for u in range(NU):
            head(u)

    nc.compile()
    return nc


def _host_prep(q, k, v):
    """Shard + transpose + fp8/fp16 prescale on host; one input map per core."""
    e4 = ml_dtypes.float8_e4m3
    in_maps = []
    ii = np.arange(128)
    mask = (ii[None, :] >= ii[:, None]).astype(np.float16)  # [sk, sq]: sq >= sk
    for c in range(NCORES):
        q8 = np.empty((NU, 128, 2, S), e4)
        k8 = np.empty((128, PAIRS, 2, S), e4)
        v16 = np.empty((128, PAIRS, NT, D), np.float16)
        for p in range(PAIRS):
            pg = c * PAIRS + p
            b, g = divmod(pg, HKV)
            tok = slice(b * S, (b + 1) * S)
            ks = (k[tok, g, :].astype(np.float64) * CK).astype(np.float32)
            k_hi = ks.astype(e4)
            k_lo = (ks - k_hi.astype(np.float32)).astype(e4)
            k8[:, p, 0, :] = k_hi.T
            k8[:, p, 1, :] = k_lo.T
            vseg = v[tok, g, :].astype(np.float16)  # [S, D]
            v16[:, p, :, :] = vseg.reshape(NT, 128, D).transpose(1, 0, 2)
            for hh in range(G):
                qs = (q[tok, g * G + hh, :].astype(np.float64) * CQ).astype(
                    np.float32)
                q_hi = qs.astype(e4)
                q_lo = (qs - q_hi.astype(np.float32)).astype(e4)
                u = p * G + hh
                q8[u, :, 0, :] = q_hi.T
                q8[u, :, 1, :] = q_lo.T
        in_maps.append({"q8": q8, "k8": k8, "v16": v16, "mask": mask})
    return in_maps


def _gather(results):
    out = np.empty((B * S, H, D), np.float32)
    for c in range(NCORES):
        o = results[c]["o"].astype(np.float32)  # [NU, 128, NT, D]
        dn = results[c]["oden"].astype(np.float32)  # [NU, 128, NT]
        o /= dn[:, :, :, None]
        for p in range(PAIRS):
            pg = c * PAIRS + p
            b, g = divmod(pg, HKV)
            for hh in range(G):
                u = p * G + hh
                # o[u, sq_in_tile, i, :] -> out[b*S + 128*i + sq_in_tile]
                out[b * S : (b + 1) * S, g * G + hh, :] = (
                    o[u].transpose(1, 0, 2).reshape(S, D))
    return out


def kernel(q, k, v, cu_seqlens_q=None, cu_seqlens_k=None, **_ignored):
    if "nc" not in _cache:
        _cache["nc"] = build_program()
    nc = _cache["nc"]
    in_maps = _host_prep(np.asarray(q), np.asarray(k), np.asarray(v))
    res = run_bass_kernel_spmd(nc, in_maps, core_ids=list(range(NCORES)))
    return _gather(res.results)


# revision 8
# speedup vs baseline: 1.0258x; 1.0258x over previous
"""Causal GQA attention (B=4, S=1024, H=16 q-heads, 4 kv-heads, D=128) on 8 trn2 cores.

Sharding: 16 (batch, kv-group) pairs -> 2 pairs/core; each pair carries 4 query
heads, so each core runs 8 independent causal-attention head-units.

Per head-unit (transposed-scores formulation, S^T[sk, sq]):
  QK^T on PE in fp8(e4m3) DoubleRow perf mode (0.5 cyc/col): q is split
    hi+lo into the two DoubleRow k-tiles (near-exact q), k single fp8 via a
    zero-stride broadcast lhsT.  Diagonal blocks accumulate a second
    DoubleRow matmul with k_lo so the high-weight near-diagonal scores are
    computed with near-fp16 accuracy.
  The matmul inputs are pre-scaled so PSUM holds 1024*log2(e)*score:
    - ACT computes exact exp via activation(Exp, scale=ln2/1024)
    - DVE computes a one-op Schraudolph exp2: int16(psum + B16) bit-cast
      to fp16 (exponent-field arithmetic), ~+-3% spread which largely
      cancels in the softmax normalization.
  Diagonal blocks: one batched DVE scalar_tensor_tensor per head fusing
    (psum + B16) * causal_mask -> int16, writing all 8 diagonal 128x128
    P-blocks through a custom strided AP.
  PV on PE in fp16; parallel 1-col ones-matmuls accumulate the softmax
    denominator into spare PSUM columns.  PSUM holds unnormalized O and den;
    ACT/DVE evacuate both to SBUF as fp16 and the o/den division runs on
    host after the gather.

PSUM budget (8 banks x 2KB): scores 2x[128,896] (4 banks), dps [128,1024]
(2 banks, diagonal blocks then reused as scratch for blocks j=4..6 + den),
po [128,8,128] (2 banks).
"""

import os
import sys

for _p in ("/opt/trn_rl_repo", "/root/.axon_site/_ro/trn_rl_repo"):
    if os.path.isdir(_p) and _p not in sys.path:
        sys.path.insert(0, _p)

from contextlib import ExitStack

import numpy as np
import ml_dtypes

import concourse.bass as bass
import concourse.tile as tile
from concourse import bacc, mybir
from concourse.ap import AP
from concourse.bass_utils import run_bass_kernel_spmd

B = 4
S = 1024
H = 16
HKV = 4
G = H // HKV
D = 128
SCALE = 0.08838834764831845
LOG2E = 1.4426950408889634
NCORES = 8
PAIRS = (B * HKV) // NCORES  # 2
NU = PAIRS * G  # 8 head-units per core
NT = S // 128  # 8

# PSUM holds 1024*log2e*score; folded into host-side q/k prescale.
CSUM = 1024.0 * LOG2E * SCALE  # ~130.577
CK = 8.0
CQ = CSUM / CK
ACT_SCALE = 1.0 / (1024.0 * LOG2E)  # recovers score from psum for exact exp
# schraudolph exp2 bias (int16/fp16 domain), geometric-mean-centered so the
# DVE exp2 approximation is unbiased relative to ACT's exact exp
B16 = 15361.0 - 1024.0 * 0.05783  # ~15301.8

FP16 = mybir.dt.float16
FP32 = mybir.dt.float32
FP8 = mybir.dt.float8e4
I16 = mybir.dt.int16
DR = mybir.MatmulPerfMode.DoubleRow

# scratch layout inside dps after the diagonal STT consumed it (fp32 cols)
SCRATCH = {4: 0, 5: 512, 6: 768}  # block j -> col offset (no bank straddle)
DEN_COL = 904
# exp engine per off-diag block j (True -> ACT exact exp, False -> DVE exp2)
ACT_BLK = [True, True, True, False, False, False, False]

_cache = {}


def build_program():
    nc = bacc.Bacc("TRN2", target_bir_lowering=False, debug=False, num_devices=NCORES)

    q8_d = nc.dram_tensor("q8", [NU, 128, 2, S], FP8, kind="ExternalInput").ap()
    k8_d = nc.dram_tensor("k8", [128, PAIRS, 2, S], FP8, kind="ExternalInput").ap()
    v16_d = nc.dram_tensor("v16", [128, PAIRS, NT, D], FP16, kind="ExternalInput").ap()
    mask_d = nc.dram_tensor("mask", [128, 128], FP16, kind="ExternalInput").ap()
    o_d = nc.dram_tensor("o", [NU, 128, NT, D], FP16, kind="ExternalOutput").ap()
    od_d = nc.dram_tensor("oden", [NU, 128, NT], FP16, kind="ExternalOutput").ap()

    with tile.TileContext(nc) as tc, ExitStack() as ctx:
        const = ctx.enter_context(tc.tile_pool(name="const", bufs=1))
        pt_pool = ctx.enter_context(tc.tile_pool(name="pt_pool", bufs=2))
        outp = ctx.enter_context(tc.tile_pool(name="outp", bufs=2))
        ps_s = ctx.enter_context(tc.tile_pool(name="ps_s", bufs=2, space="PSUM"))
        ps_d = ctx.enter_context(tc.tile_pool(name="ps_d", bufs=1, space="PSUM"))
        ps_o = ctx.enter_context(tc.tile_pool(name="ps_o", bufs=1, space="PSUM"))

        q8s = const.tile([128, NU, 2, S], FP8)
        k8s = const.tile([128, PAIRS, 2, S], FP8)
        v16s = const.tile([128, PAIRS, NT, D], FP16)
        mask16 = const.tile([128, 128], FP16)
        ones16 = const.tile([128, 1], FP16)

        nc.sync.dma_start(out=mask16, in_=mask_d)
        nc.sync.dma_start(out=k8s, in_=k8_d)
        nc.vector.memset(ones16, 1.0)
        nc.sync.dma_start(out=v16s[:, 0], in_=v16_d[:, 0])
        nc.sync.dma_start(out=q8s[:, 0], in_=q8_d[0])
        nc.sync.dma_start(out=q8s[:, 1], in_=q8_d[1])
        nc.sync.dma_start(out=v16s[:, 1], in_=v16_d[:, 1])
        for u in range(2, NU):
            nc.sync.dma_start(out=q8s[:, u], in_=q8_d[u])

        def k_lhsT(pair, j, lo):
            # [128, 2, 128] DoubleRow stationary: k block j broadcast over the
            # two k-tiles (matching the hi/lo q streams); lo picks k_hi/k_lo.
            blk = k8s[:, pair, lo, 128 * j : 128 * j + 128]
            return blk.rearrange("p (o k) -> p o k", o=1).broadcast_to([128, 2, 128])

        def head(u):
            pair, h = divmod(u, G)
            pt = pt_pool.tile([128, NT, S], FP16, tag="pt", name=f"pt_{u}")
            ob = outp.tile([128, NT, D], FP16, tag="ob", name=f"ob_{u}")
            obden = outp.tile([128, NT], FP16, tag="obd", name=f"obd_{u}")
            dps = ps_d.tile([128, 1024], FP32, tag="dps", name=f"dps_{u}")
            po = ps_o.tile([128, NT, D], FP32, tag="po", name=f"po_{u}")
            dpsv = dps.rearrange("p (j c) -> p j c", j=NT)

            qrhs = q8s[:, u]  # [128, 2, S]

            # --- diagonal QK blocks -> dps, with k_lo correction ---
            for j in range(NT):
                sq = 128 * j
                for lo in (0, 1):
                    nc.tensor.matmul(
                        dpsv[:, j, :], lhsT=k_lhsT(pair, j, lo),
                        rhs=qrhs[:, :, sq : sq + 128],
                        start=(lo == 0), stop=(lo == 1), perf_mode=DR,
                    )
            # one fused DVE op: (dps + B16) * mask -> int16 -> all 8 diagonal
            # P blocks via strided AP pt[:, j, 128j:128j+128]
            pt_i16 = pt.bitcast(I16)
            base = pt_i16[:, 0, 0:128]
            diag_out = AP(base.tensor, base.offset,
                          [list(base.ap[0]), [S + 128, NT], [1, 128]])
            mask_b = mask16.rearrange("p (o c) -> p o c", o=1).broadcast_to(
                [128, NT, 128])
            nc.vector.scalar_tensor_tensor(
                out=diag_out, in0=dpsv, scalar=B16, in1=mask_b,
                op0=mybir.AluOpType.add, op1=mybir.AluOpType.mult,
            )

            def pv(i):
                for j in range(i + 1):
                    lhsT = pt[:, j, 128 * i : 128 * i + 128]
                    nc.tensor.matmul(
                        po[:, i, :], lhsT=lhsT, rhs=v16s[:, pair, j, :],
                        start=(j == 0), stop=(j == i),
                    )
                    nc.tensor.matmul(
                        dps[:, DEN_COL + i : DEN_COL + i + 1], lhsT=lhsT,
                        rhs=ones16, start=(j == 0), stop=(j == i),
                    )

            def exp_piece(j, psrc, q0, w):
                if ACT_BLK[j]:
                    nc.scalar.activation(
                        out=pt[:, j, q0 : q0 + w], in_=psrc,
                        func=mybir.ActivationFunctionType.Exp, scale=ACT_SCALE,
                    )
                else:
                    nc.vector.tensor_scalar(
                        out=pt_i16[:, j, q0 : q0 + w], in0=psrc,
                        scalar1=B16, scalar2=None, op0=mybir.AluOpType.add,
                    )

            # --- off-diag blocks j=0..3 through ps_s; j=4..6 through dps ---
            # software pipeline: QK of block j+1 is emitted before pv(j) so the
            # in-order PE queue always has matmul work while ACT/DVE exp block j.
            def qk_off(j):
                q0 = 128 * (j + 1)
                w = S - q0
                if j < 4:
                    sp = ps_s.tile([128, 896], FP32, tag="sp", name=f"sp_{u}_{j}")
                    dst = sp[:, 0:w]
                else:
                    dst = dps[:, SCRATCH[j] : SCRATCH[j] + w]
                for c in range(0, w, 256):
                    cw = min(256, w - c)
                    nc.tensor.matmul(
                        dst[:, c : c + cw], lhsT=k_lhsT(pair, j, 0),
                        rhs=qrhs[:, :, q0 + c : q0 + c + cw],
                        start=True, stop=True, perf_mode=DR,
                    )
                return dst, q0, w

            pieces = {}
            pieces[0] = qk_off(0)
            exp_piece(0, *pieces[0])
            pieces[1] = qk_off(1)
            pv(0)
            exp_piece(1, *pieces[1])
            for j in range(2, NT - 1):
                pieces[j] = qk_off(j)
                pv(j - 1)
                exp_piece(j, *pieces[j])
                if j == 5:
                    # rowtiles 0-3 complete: ACT evacuates the first half of po
                    nc.scalar.copy(ob[:, 0:4, :], po[:, 0:4, :])
            pv(6)
            pv(7)

            # --- evacuate second half + den; host divides o by den ---
            nc.vector.tensor_copy(obden, dps[:, DEN_COL : DEN_COL + NT])
            nc.vector.tensor_copy(ob[:, 4:8, :], po[:, 4:8, :])
            nc.sync.dma_start(out=o_d[u], in_=ob)
            nc.sync.dma_start(out=od_d[u], in_=obden)

        for u in range(NU):
            head(u)

    nc.compile()
    return nc


def _host_prep(q, k, v):
    """Shard + transpose + fp8/fp16 prescale on host; one input map per core."""
    e4 = ml_dtypes.float8_e4m3
    in_maps = []
    ii = np.arange(128)
    mask = (ii[None, :] >= ii[:, None]).astype(np.float16)  # [sk, sq]: sq >= sk
    for c in range(NCORES):
        q8 = np.empty((NU, 128, 2, S), e4)
        k8 = np.empty((128, PAIRS, 2, S), e4)
        v16 = np.empty((128, PAIRS, NT, D), np.float16)
        for p in range(PAIRS):
            pg = c * PAIRS + p
            b, g = divmod(pg, HKV)
            tok = slice(b * S, (b + 1) * S)
            ks = (k[tok, g, :].astype(np.float64) * CK).astype(np.float32)
            k_hi = ks.astype(e4)
            k_lo = (ks - k_hi.astype(np.float32)).astype(e4)
            k8[:, p, 0, :] = k_hi.T
            k8[:, p, 1, :] = k_lo.T
            vseg = v[tok, g, :].astype(np.float16)  # [S, D]
            v16[:, p, :, :] = vseg.reshape(NT, 128, D).transpose(1, 0, 2)
            for hh in range(G):
                qs = (q[tok, g * G + hh, :].astype(np.float64) * CQ).astype(
                    np.float32)
                q_hi = qs.astype(e4)
                q_lo = (qs - q_hi.astype(np.float32)).astype(e4)
                u = p * G + hh
                q8[u, :, 0, :] = q_hi.T
                q8[u, :, 1, :] = q_lo.T
        in_maps.append({"q8": q8, "k8": k8, "v16": v16, "mask": mask})
    return in_maps


def _gather(results):
    out = np.empty((B * S, H, D), np.float32)
    for c in range(NCORES):
        o = results[c]["o"].astype(np.float32)  # [NU, 128, NT, D]
        dn = results[c]["oden"].astype(np.float32)  # [NU, 128, NT]
        o /= dn[:, :, :, None]
        for p in range(PAIRS):
            pg = c * PAIRS + p
            b, g = divmod(pg, HKV)
            for hh in range(G):
                u = p * G + hh
                # o[u, sq_in_tile, i, :] -> out[b*S + 128*i + sq_in_tile]
                out[b * S : (b + 1) * S, g * G + hh, :] = (
                    o[u].transpose(1, 0, 2).reshape(S, D))
    return out


def kernel(q, k, v, cu_seqlens_q=None, cu_seqlens_k=None, **_ignored):
    if "nc" not in _cache:
        _cache["nc"] = build_program()
    nc = _cache["nc"]
    in_maps = _host_prep(np.asarray(q), np.asarray(k), np.asarray(v))
    res = run_bass_kernel_spmd(nc, in_maps, core_ids=list(range(NCORES)))
    return _gather(res.results)
